# revision 30
# baseline (speedup 1.0000x reference)
"""Trainium2 Bass kernel for nn_CustomLoss_84043920048360.

Data-parallel over batch: 8 NeuronCores x 4 batches each, no collectives.

The loss reduces to per-batch segment-sums over positions s:
  Q[j, c]      = sum_{s: target[s]==j} x[s, c]
  counts[j, c] = sum_{s: target[s]==j} [argmax_c' x[s, c'] == c]
  sumexp[s]    = sum_c exp(x[s, c])

Device pipeline (per 128-position chunk, all-bf16 matmul path):
  - x ships as bf16, onehot(target) is built on the host (np.eye gather)
    and ships as bf16 -> total input DMA 16.8 MB/core.
  - DVE: rowmax (f32 out) + 6/8 argmax-onehot compares (bf16 4x mode);
    GPSIMD takes 2/8 compare chunks and the x copy into the matmul rhs
    (uint32-bitcast so the bf16 move halves its element count).
  - TensorE: ONE bf16 matmul per chunk, lhsT=onehot(target),
    rhs=[x | onehot(argmax)] (N=256), accumulated over 64 chunks in PSUM
    -> [Q | counts]. Plus a bf16 transpose of x into PSUM and an N=1
    ones-matmul on exp(xT) that computes sumexp on the PE.
  - ACT: exp (PSUM->SBUF, one op per 8 chunks) + PSUM evacuation copies.
The host does lse=log(sumexp), bincounts, mode=argmax(counts) (exact
tie-break), the cipher/nll formulas in float64, and the final combine.

Accuracy: counts/mode are exact except for bf16-argmax ties (~1.8% of
rows, washes out to ~2e-4 on the final scalar); Q/lse carry bf16 noise
(~1e-4). Measured end-to-end relative error vs the f32 reference: 1.6e-4.

Position mapping within a 1024-position block: s = it*1024 + p*8 + g
(p = SBUF partition, g = chunk-in-iter) so each partition's DMA is one
contiguous 2 KiB run.
"""

import numpy as np
import ml_dtypes

B, S, C = 32, 8192, 128
NCORES = 8
B_LOC = B // NCORES          # 4 batches per core
G = 8                        # chunks per iteration
CHUNK = 128                  # positions per chunk (matmul K)
ITERS = S // (G * CHUNK)     # 8 iterations per batch
NCHUNK = S // CHUNK          # 64 chunks per batch

_cache = {}


def _build(b_loc=B_LOC, iters=ITERS, n_pool=2, wbufs=4, pbufs=2, esplit=1):
    import concourse.bacc as bacc
    import concourse.tile as tile
    from concourse import mybir

    f32 = mybir.dt.float32
    bf16 = mybir.dt.bfloat16
    u32 = mybir.dt.uint32
    s_loc = iters * G * CHUNK

    nc = bacc.Bacc(
        "TRN2", target_bir_lowering=False, debug=False, num_devices=NCORES
    )
    pred = nc.dram_tensor("predicted", [b_loc, s_loc, C], bf16, kind="ExternalInput")
    oht_in = nc.dram_tensor("oht_bf16", [b_loc, s_loc, C], bf16, kind="ExternalInput")
    ident = nc.dram_tensor("ident_bf16", [128, 128], bf16, kind="ExternalInput")
    q_out = nc.dram_tensor("q_out", [b_loc, 128, 256], f32, kind="ExternalOutput")
    se_out = nc.dram_tensor(
        "se_out", [b_loc, 128, iters * G], f32, kind="ExternalOutput"
    )

    # s = it*(G*128) + p*G + g
    pv = pred.ap().rearrange("b (i p g) c -> b i p g c", i=iters, p=128, g=G)
    ov = oht_in.ap().rearrange("b (i p g) c -> b i p g c", i=iters, p=128, g=G)

    AX = mybir.AxisListType.X
    EQ = mybir.AluOpType.is_equal

    with tile.TileContext(nc) as tc:
        with (
            tc.tile_pool(name="consts", bufs=1) as consts,
            tc.tile_pool(name="work", bufs=wbufs) as work,
            tc.tile_pool(name="psum", bufs=pbufs, space="PSUM") as psum,
        ):
            ident_sb = consts.tile([128, 128], bf16)
            nc.sync.dma_start(ident_sb[:], ident.ap())
            ones_sb = consts.tile([128, 1], bf16)
            nc.vector.memset(ones_sb[:], 1.0)

            for b in range(b_loc):
                ps = psum.tile([128, 256], f32, tag="ps")
                se_ps = psum.tile([128, iters * G], f32, tag="se")
                for it in range(iters):
                    rhs = work.tile([128, G, 256], bf16, tag="rhs")
                    oht = work.tile([128, G, 128], bf16, tag="oht")
                    eT = work.tile([128, G, 128], bf16, tag="e")
                    rmax = work.tile([128, G], f32, tag="rmax")
                    xb_t = work.tile([128, G, 128], bf16, tag="xbt")
                    xT = psum.tile([128, G, 128], bf16, tag="xT")

                    nc.sync.dma_start(xb_t[:], pv[b, it])
                    nc.sync.dma_start(oht[:], ov[b, it])
                    # x copy into the matmul rhs (bf16 moved as u32)
                    nc.gpsimd.tensor_copy(
                        rhs[:, :, 0:128].bitcast(u32), xb_t[:].bitcast(u32)
                    )
                    # rowmax over classes (bf16 in, f32 out)
                    nc.vector.reduce_max(rmax[:], xb_t[:], axis=AX)
                    for g in range(G):
                        # onehot(argmax): bf16 compare, exact 0/1 out
                        eng = nc.gpsimd if g < n_pool else nc.vector
                        eng.tensor_scalar(
                            out=rhs[:, g, 128:256],
                            in0=xb_t[:, g, :],
                            scalar1=rmax[:, g : g + 1],
                            scalar2=None,
                            op0=EQ,
                        )
                        # transpose x chunk into PSUM: xT[c, s]
                        nc.tensor.transpose(
                            xT[:, g, :], rhs[:, g, 0:128], ident_sb[:]
                        )
                    # exp on the transposed tile (PSUM -> SBUF)
                    EH = G // esplit
                    for h in range(esplit):
                        nc.scalar.activation(
                            eT[:, h * EH : (h + 1) * EH, :],
                            xT[:, h * EH : (h + 1) * EH, :],
                            mybir.ActivationFunctionType.Exp,
                        )
                    for g in range(G):
                        # sumexp[s] = eT.T @ ones = row sums (PE, N=1)
                        nc.tensor.matmul(
                            se_ps[:, it * G + g : it * G + g + 1],
                            eT[:, g, :],
                            ones_sb[:],
                            start=True,
                            stop=True,
                        )
                        # segment-sum matmul: psum += oht.T @ [x|onehot(am)]
                        nc.tensor.matmul(
                            ps[:],
                            oht[:, g, :],
                            rhs[:, g, :],
                            start=(it == 0 and g == 0),
                            stop=(it == iters - 1 and g == G - 1),
                        )
                q_sb = work.tile([128, 256], f32, tag="q")
                nc.scalar.copy(q_sb[:], ps[:])
                nc.sync.dma_start(q_out.ap()[b], q_sb[:])
                se_sb = work.tile([128, iters * G], f32, tag="sesb")
                nc.scalar.copy(se_sb[:], se_ps[:])
                nc.sync.dma_start(se_out.ap()[b], se_sb[:])

    nc.compile()
    return nc


def _get_nc(b_loc=B_LOC, iters=ITERS):
    key = (b_loc, iters)
    if key not in _cache:
        _cache[key] = _build(b_loc, iters)
    return _cache[key]


_BF16 = ml_dtypes.bfloat16
_IDENT = np.eye(128).astype(_BF16)
_EYE = np.eye(128).astype(_BF16)
last_results = None


def _run_device(predicted, target):
    """predicted [B,S,C] f32, target [B,S] int -> (q [B,128,256], se [B,S]) float64"""
    from concourse.bass_utils import run_bass_kernel_spmd

    nc = _get_nc()
    xb = predicted.astype(_BF16)
    ohtb = _EYE[target.astype(np.int64)]
    in_maps = []
    for core in range(NCORES):
        b0 = core * B_LOC
        in_maps.append(
            {
                "predicted": np.ascontiguousarray(xb[b0 : b0 + B_LOC]),
                "oht_bf16": np.ascontiguousarray(ohtb[b0 : b0 + B_LOC]),
                "ident_bf16": _IDENT,
            }
        )
    global last_results
    last_results = run_bass_kernel_spmd(
        nc, in_maps, core_ids=list(range(NCORES))
    )
    q = np.concatenate([r["q_out"] for r in last_results.results], axis=0)
    se = np.concatenate([r["se_out"] for r in last_results.results], axis=0)
    # se[b, p, it*G+g] -> sumexp[b, s] with s = it*(G*128) + p*G + g
    se = (
        se.reshape(B, 128, ITERS, G)
        .transpose(0, 2, 1, 3)
        .reshape(B, S)
    )
    return q.astype(np.float64), se.astype(np.float64)


def kernel(predicted, target):
    predicted = np.asarray(predicted)
    target = np.asarray(target)
    in_dtype = predicted.dtype
    q, se = _run_device(predicted.astype(np.float32, copy=False), target)

    total_cipher = 0.0
    total_nz = 0
    total_gather = 0.0
    for b in range(B):
        Q = q[b, :, 0:128]          # [j, c] segment sums of x (bf16 inputs)
        counts = q[b, :, 128:256]   # [j, c] argmax histogram
        t_b = target[b].astype(np.int64)
        lse = np.log(se[b])
        n_eq = np.bincount(t_b, minlength=C).astype(np.float64)
        Lt = np.bincount(t_b, weights=lse, minlength=C)
        L = lse.sum()
        mode = np.argmax(counts, axis=1)
        P = Q.sum(axis=0)
        Qg = Q[np.arange(C), mode]
        Pg = P[mode]
        sum_all = L - Pg
        sum_eq = Lt - Qg
        sum_ne = sum_all - sum_eq
        ne_cnt = S - n_eq
        eq_mean = sum_eq / np.maximum(n_eq, 1.0)
        ne_mean = sum_ne / np.maximum(ne_cnt, 1.0)
        inv_ne = np.where(ne_cnt > 0, 1.0 / np.maximum(ne_mean, 1e-30), 0.0)
        cipher = np.where(n_eq > 0, 0.5 * eq_mean + 0.5 * inv_ne, 0.0)
        total_cipher += cipher.sum()
        total_nz += int((cipher != 0).sum())
        total_gather += Q[np.arange(C), np.arange(C)].sum()

    cipher_mean = total_cipher / max(total_nz, 1)
    nll = -total_gather / (B * S)
    out = 0.5 * cipher_mean + 0.5 * nll
    out_dtype = in_dtype if in_dtype in (np.float32, np.float64) else np.float32
    return np.asarray(out, dtype=out_dtype)


# revision 34
# speedup vs baseline: 1.0482x; 1.0482x over previous
"""Trainium2 Bass kernel for nn_CustomLoss_84043920048360.

Data-parallel over batch: 8 NeuronCores x 4 batches each, no collectives.

The loss reduces to per-batch segment-sums over positions s:
  Q[j, c]      = sum_{s: target[s]==j} x[s, c]
  counts[j, c] = sum_{s: target[s]==j} [argmax_c' x[s, c'] == c]
  sumexp[s]    = sum_c exp(x[s, c])

Device pipeline (per 128-position chunk, all-bf16 matmul path):
  - x ships as bf16, onehot(target) is built on the host (np.eye gather)
    and ships as bf16 -> total input DMA 16.8 MB/core.
  - DVE: rowmax (f32 out) + 6/8 argmax-onehot compares (bf16 4x mode);
    GPSIMD takes 2/8 compare chunks and the x copy into the matmul rhs
    (uint32-bitcast so the bf16 move halves its element count).
  - TensorE: ONE bf16 matmul per chunk, lhsT=onehot(target),
    rhs=[x | onehot(argmax)] (N=256), accumulated over 64 chunks in PSUM
    -> [Q | counts]. Plus a bf16 transpose of x into PSUM and an N=1
    ones-matmul on exp(xT) that computes sumexp on the PE.
  - ACT: exp (PSUM->SBUF, one op per 8 chunks) + PSUM evacuation copies.
The host does lse=log(sumexp), bincounts, mode=argmax(counts) (exact
tie-break), the cipher/nll formulas in float64, and the final combine.

Accuracy: counts/mode are exact except for bf16-argmax ties (~1.8% of
rows, washes out to ~2e-4 on the final scalar); Q/lse carry bf16 noise
(~1e-4). Measured end-to-end relative error vs the f32 reference: 1.6e-4.

Position mapping within a 1024-position block: s = it*1024 + p*8 + g
(p = SBUF partition, g = chunk-in-iter) so each partition's DMA is one
contiguous 2 KiB run.
"""

import numpy as np
import ml_dtypes

B, S, C = 32, 8192, 128
NCORES = 8
B_LOC = B // NCORES          # 4 batches per core
G = 8                        # chunks per iteration
CHUNK = 128                  # positions per chunk (matmul K)
ITERS = S // (G * CHUNK)     # 8 iterations per batch
NCHUNK = S // CHUNK          # 64 chunks per batch

_cache = {}


def _build(b_loc=B_LOC, iters=ITERS, n_pool=2, wbufs=4, pbufs=2, esplit=1, two_mm=False, fuse=True, fuse2=False, xbufs=0, rsplit=1):
    import concourse.bacc as bacc
    import concourse.tile as tile
    from concourse import mybir

    f32 = mybir.dt.float32
    bf16 = mybir.dt.bfloat16
    u32 = mybir.dt.uint32
    s_loc = iters * G * CHUNK

    nc = bacc.Bacc(
        "TRN2", target_bir_lowering=False, debug=False, num_devices=NCORES
    )
    pred = nc.dram_tensor("predicted", [b_loc, s_loc, C], bf16, kind="ExternalInput")
    oht_in = nc.dram_tensor("oht_bf16", [b_loc, s_loc, C], bf16, kind="ExternalInput")
    ident = nc.dram_tensor("ident_bf16", [128, 128], bf16, kind="ExternalInput")
    q_out = nc.dram_tensor("q_out", [b_loc, 128, 256], f32, kind="ExternalOutput")
    se_out = nc.dram_tensor(
        "se_out", [b_loc, 128, iters * G], f32, kind="ExternalOutput"
    )

    # s = it*(G*128) + p*G + g
    pv = pred.ap().rearrange("b (i p g) c -> b i p g c", i=iters, p=128, g=G)
    ov = oht_in.ap().rearrange("b (i p g) c -> b i p g c", i=iters, p=128, g=G)

    AX = mybir.AxisListType.X
    EQ = mybir.AluOpType.is_equal

    with tile.TileContext(nc) as tc:
        with (
            tc.tile_pool(name="consts", bufs=1) as consts,
            tc.tile_pool(name="work", bufs=wbufs) as work,
            tc.tile_pool(name="psum", bufs=pbufs, space="PSUM") as psum,
            tc.tile_pool(name="psumx", bufs=(xbufs or pbufs), space="PSUM") as psumx,
        ):
            ident_sb = consts.tile([128, 128], bf16)
            nc.sync.dma_start(ident_sb[:], ident.ap())
            ones_sb = consts.tile([128, 1], bf16)
            nc.vector.memset(ones_sb[:], 1.0)

            for b in range(b_loc):
                ps = psum.tile([128, 256], f32, tag="ps")
                se_ps = psum.tile([128, iters * G], f32, tag="se")
                for it in range(iters):
                    if two_mm:
                        oham_t = work.tile([128, G, 128], bf16, tag="oham")
                        rhs = None
                    else:
                        rhs = work.tile([128, G, 256], bf16, tag="rhs")
                    oht = work.tile([128, G, 128], bf16, tag="oht")
                    eT = work.tile([128, G, 128], bf16, tag="e")
                    rmax = work.tile([128, G], f32, tag="rmax")
                    xb_t = work.tile([128, G, 128], bf16, tag="xbt")
                    xT = psumx.tile([128, G, 128], bf16, tag="xT")

                    nc.sync.dma_start(xb_t[:], pv[b, it])
                    nc.sync.dma_start(oht[:], ov[b, it])
                    if not two_mm:
                        # x copy into the matmul rhs (bf16 moved as u32)
                        nc.gpsimd.tensor_copy(
                            rhs[:, :, 0:128].bitcast(u32), xb_t[:].bitcast(u32)
                        )
                    # rowmax over classes (bf16 in, f32 out)
                    RH = G // rsplit
                    for h in range(rsplit):
                        nc.vector.reduce_max(
                            rmax[:, h * RH : (h + 1) * RH],
                            xb_t[:, h * RH : (h + 1) * RH, :],
                            axis=AX,
                        )
                    for g in range(G):
                        # onehot(argmax): bf16 compare, exact 0/1 out
                        eng = nc.gpsimd if g < n_pool else nc.vector
                        eng.tensor_scalar(
                            out=(oham_t[:, g, :] if two_mm else rhs[:, g, 128:256]),
                            in0=xb_t[:, g, :],
                            scalar1=rmax[:, g : g + 1],
                            scalar2=None,
                            op0=EQ,
                        )
                        # transpose x chunk into PSUM: xT[c, s]
                        nc.tensor.transpose(
                            xT[:, g, :], xb_t[:, g, :], ident_sb[:]
                        )
                        if fuse2:
                            nc.tensor.matmul(
                                ps[:],
                                oht[:, g, :],
                                rhs[:, g, :],
                                start=(it == 0 and g == 0),
                                stop=(it == iters - 1 and g == G - 1),
                            )
                    if fuse and not fuse2:
                        for g in range(G):
                            nc.tensor.matmul(
                                ps[:],
                                oht[:, g, :],
                                rhs[:, g, :],
                                start=(it == 0 and g == 0),
                                stop=(it == iters - 1 and g == G - 1),
                            )
                    # exp on the transposed tile (PSUM -> SBUF)
                    EH = G // esplit
                    for h in range(esplit):
                        nc.scalar.activation(
                            eT[:, h * EH : (h + 1) * EH, :],
                            xT[:, h * EH : (h + 1) * EH, :],
                            mybir.ActivationFunctionType.Exp,
                        )
                    for g in range(G):
                        # sumexp[s] = eT.T @ ones = row sums (PE, N=1)
                        nc.tensor.matmul(
                            se_ps[:, it * G + g : it * G + g + 1],
                            eT[:, g, :],
                            ones_sb[:],
                            start=True,
                            stop=True,
                        )
                        # segment-sum matmul: psum += oht.T @ [x|onehot(am)]
                        if fuse or fuse2:
                            pass
                        elif two_mm:
                            nc.tensor.matmul(
                                ps[:, 0:128],
                                oht[:, g, :],
                                xb_t[:, g, :],
                                start=(it == 0 and g == 0),
                                stop=(it == iters - 1 and g == G - 1),
                            )
                            nc.tensor.matmul(
                                ps[:, 128:256],
                                oht[:, g, :],
                                oham_t[:, g, :],
                                start=(it == 0 and g == 0),
                                stop=(it == iters - 1 and g == G - 1),
                            )
                        else:
                            nc.tensor.matmul(
                                ps[:],
                                oht[:, g, :],
                                rhs[:, g, :],
                                start=(it == 0 and g == 0),
                                stop=(it == iters - 1 and g == G - 1),
                            )
                q_sb = work.tile([128, 256], f32, tag="q")
                nc.scalar.copy(q_sb[:], ps[:])
                nc.sync.dma_start(q_out.ap()[b], q_sb[:])
                se_sb = work.tile([128, iters * G], f32, tag="sesb")
                nc.scalar.copy(se_sb[:], se_ps[:])
                nc.sync.dma_start(se_out.ap()[b], se_sb[:])

    nc.compile()
    return nc


def _get_nc(b_loc=B_LOC, iters=ITERS):
    key = (b_loc, iters)
    if key not in _cache:
        _cache[key] = _build(b_loc, iters)
    return _cache[key]


_BF16 = ml_dtypes.bfloat16
_IDENT = np.eye(128).astype(_BF16)
_EYE = np.eye(128).astype(_BF16)
last_results = None


def _run_device(predicted, target):
    """predicted [B,S,C] f32, target [B,S] int -> (q [B,128,256], se [B,S]) float64"""
    from concourse.bass_utils import run_bass_kernel_spmd

    nc = _get_nc()
    xb = predicted.astype(_BF16)
    ohtb = _EYE[target.astype(np.int64)]
    in_maps = []
    for core in range(NCORES):
        b0 = core * B_LOC
        in_maps.append(
            {
                "predicted": np.ascontiguousarray(xb[b0 : b0 + B_LOC]),
                "oht_bf16": np.ascontiguousarray(ohtb[b0 : b0 + B_LOC]),
                "ident_bf16": _IDENT,
            }
        )
    global last_results
    last_results = run_bass_kernel_spmd(
        nc, in_maps, core_ids=list(range(NCORES))
    )
    q = np.concatenate([r["q_out"] for r in last_results.results], axis=0)
    se = np.concatenate([r["se_out"] for r in last_results.results], axis=0)
    # se[b, p, it*G+g] -> sumexp[b, s] with s = it*(G*128) + p*G + g
    se = (
        se.reshape(B, 128, ITERS, G)
        .transpose(0, 2, 1, 3)
        .reshape(B, S)
    )
    return q.astype(np.float64), se.astype(np.float64)


def kernel(predicted, target):
    predicted = np.asarray(predicted)
    target = np.asarray(target)
    in_dtype = predicted.dtype
    q, se = _run_device(predicted.astype(np.float32, copy=False), target)

    total_cipher = 0.0
    total_nz = 0
    total_gather = 0.0
    for b in range(B):
        Q = q[b, :, 0:128]          # [j, c] segment sums of x (bf16 inputs)
        counts = q[b, :, 128:256]   # [j, c] argmax histogram
        t_b = target[b].astype(np.int64)
        lse = np.log(se[b])
        n_eq = np.bincount(t_b, minlength=C).astype(np.float64)
        Lt = np.bincount(t_b, weights=lse, minlength=C)
        L = lse.sum()
        mode = np.argmax(counts, axis=1)
        P = Q.sum(axis=0)
        Qg = Q[np.arange(C), mode]
        Pg = P[mode]
        sum_all = L - Pg
        sum_eq = Lt - Qg
        sum_ne = sum_all - sum_eq
        ne_cnt = S - n_eq
        eq_mean = sum_eq / np.maximum(n_eq, 1.0)
        ne_mean = sum_ne / np.maximum(ne_cnt, 1.0)
        inv_ne = np.where(ne_cnt > 0, 1.0 / np.maximum(ne_mean, 1e-30), 0.0)
        cipher = np.where(n_eq > 0, 0.5 * eq_mean + 0.5 * inv_ne, 0.0)
        total_cipher += cipher.sum()
        total_nz += int((cipher != 0).sum())
        total_gather += Q[np.arange(C), np.arange(C)].sum()

    cipher_mean = total_cipher / max(total_nz, 1)
    nll = -total_gather / (B * S)
    out = 0.5 * cipher_mean + 0.5 * nll
    out_dtype = in_dtype if in_dtype in (np.float32, np.float64) else np.float32
    return np.asarray(out, dtype=out_dtype)


# revision 37
# speedup vs baseline: 1.0747x; 1.0253x over previous
"""Trainium2 Bass kernel for nn_CustomLoss_84043920048360.

Data-parallel over batch: 8 NeuronCores x 4 batches each, no collectives.

The loss reduces to per-batch segment-sums over positions s:
  Q[j, c]      = sum_{s: target[s]==j} x[s, c]
  counts[j, c] = sum_{s: target[s]==j} [argmax_c' x[s, c'] == c]
  sumexp[s]    = sum_c exp(x[s, c])

Device pipeline (per 128-position chunk, all-bf16 matmul path):
  - x ships as bf16, onehot(target) is built on the host (np.eye gather)
    and ships as bf16 -> total input DMA 16.8 MB/core.
  - DVE: rowmax (f32 out) + 6/8 argmax-onehot compares (bf16 4x mode);
    GPSIMD takes 2/8 compare chunks and the x copy into the matmul rhs
    (uint32-bitcast so the bf16 move halves its element count).
  - TensorE: ONE bf16 matmul per chunk, lhsT=onehot(target),
    rhs=[x | onehot(argmax)] (N=256), accumulated over 64 chunks in PSUM
    -> [Q | counts]. Plus a bf16 transpose of x into PSUM and an N=1
    ones-matmul on exp(xT) that computes sumexp on the PE.
  - ACT: exp (PSUM->SBUF, one op per 8 chunks) + PSUM evacuation copies.
The host does lse=log(sumexp), bincounts, mode=argmax(counts) (exact
tie-break), the cipher/nll formulas in float64, and the final combine.

Accuracy: counts/mode are exact except for bf16-argmax ties (~1.8% of
rows, washes out to ~2e-4 on the final scalar); Q/lse carry bf16 noise
(~1e-4). Measured end-to-end relative error vs the f32 reference: 1.6e-4.

Position mapping within a 1024-position block: s = it*1024 + p*8 + g
(p = SBUF partition, g = chunk-in-iter) so each partition's DMA is one
contiguous 2 KiB run.
"""

import numpy as np
import ml_dtypes

B, S, C = 32, 8192, 128
NCORES = 8
B_LOC = B // NCORES          # 4 batches per core
G = 16                       # chunks per iteration
CHUNK = 128                  # positions per chunk (matmul K)
ITERS = S // (G * CHUNK)     # 8 iterations per batch
NCHUNK = S // CHUNK          # 64 chunks per batch

_cache = {}


def _build(b_loc=B_LOC, iters=ITERS, n_pool=3, wbufs=4, pbufs=2, esplit=1, two_mm=False, fuse=True, fuse2=False, xbufs=0, rsplit=1, rmax_bf=False, g_ovr=0):
    import concourse.bacc as bacc
    import concourse.tile as tile
    from concourse import mybir

    f32 = mybir.dt.float32
    bf16 = mybir.dt.bfloat16
    u32 = mybir.dt.uint32
    G_ = g_ovr or G
    s_loc = iters * G_ * CHUNK

    nc = bacc.Bacc(
        "TRN2", target_bir_lowering=False, debug=False, num_devices=NCORES
    )
    pred = nc.dram_tensor("predicted", [b_loc, s_loc, C], bf16, kind="ExternalInput")
    oht_in = nc.dram_tensor("oht_bf16", [b_loc, s_loc, C], bf16, kind="ExternalInput")
    ident = nc.dram_tensor("ident_bf16", [128, 128], bf16, kind="ExternalInput")
    q_out = nc.dram_tensor("q_out", [b_loc, 128, 256], f32, kind="ExternalOutput")
    se_out = nc.dram_tensor(
        "se_out", [b_loc, 128, iters * G_], f32, kind="ExternalOutput"
    )

    # s = it*(G*128) + p*G + g
    pv = pred.ap().rearrange("b (i p g) c -> b i p g c", i=iters, p=128, g=G_)
    ov = oht_in.ap().rearrange("b (i p g) c -> b i p g c", i=iters, p=128, g=G_)

    AX = mybir.AxisListType.X
    EQ = mybir.AluOpType.is_equal

    with tile.TileContext(nc) as tc:
        with (
            tc.tile_pool(name="consts", bufs=1) as consts,
            tc.tile_pool(name="work", bufs=wbufs) as work,
            tc.tile_pool(name="psum", bufs=pbufs, space="PSUM") as psum,
            tc.tile_pool(name="psumx", bufs=(xbufs or pbufs), space="PSUM") as psumx,
        ):
            ident_sb = consts.tile([128, 128], bf16)
            nc.sync.dma_start(ident_sb[:], ident.ap())
            ones_sb = consts.tile([128, 1], bf16)
            nc.vector.memset(ones_sb[:], 1.0)

            for b in range(b_loc):
                ps = psum.tile([128, 256], f32, tag="ps")
                se_ps = psum.tile([128, iters * G_], f32, tag="se")
                for it in range(iters):
                    if two_mm:
                        oham_t = work.tile([128, G_, 128], bf16, tag="oham")
                        rhs = None
                    else:
                        rhs = work.tile([128, G_, 256], bf16, tag="rhs")
                    oht = work.tile([128, G_, 128], bf16, tag="oht")
                    eT = work.tile([128, G_, 128], bf16, tag="e")
                    rmax = work.tile([128, G_], bf16 if rmax_bf else f32, tag="rmax")
                    if rmax_bf:
                        rmax32 = work.tile([128, G_], f32, tag="rmax32")
                    xb_t = work.tile([128, G_, 128], bf16, tag="xbt")
                    xT = psumx.tile([128, G_, 128], bf16, tag="xT")

                    nc.sync.dma_start(xb_t[:], pv[b, it])
                    nc.sync.dma_start(oht[:], ov[b, it])
                    if not two_mm:
                        # x copy into the matmul rhs (bf16 moved as u32)
                        nc.gpsimd.tensor_copy(
                            rhs[:, :, 0:128].bitcast(u32), xb_t[:].bitcast(u32)
                        )
                    # rowmax over classes (bf16 in, f32 out)
                    RH = G_ // rsplit
                    for h in range(rsplit):
                        nc.vector.reduce_max(
                            rmax[:, h * RH : (h + 1) * RH],
                            xb_t[:, h * RH : (h + 1) * RH, :],
                            axis=AX,
                        )
                    if rmax_bf:
                        nc.vector.tensor_copy(rmax32[:], rmax[:])
                    for g in range(G_):
                        # onehot(argmax): bf16 compare, exact 0/1 out
                        eng = nc.gpsimd if g < n_pool else nc.vector
                        eng.tensor_scalar(
                            out=(oham_t[:, g, :] if two_mm else rhs[:, g, 128:256]),
                            in0=xb_t[:, g, :],
                            scalar1=(rmax32 if rmax_bf else rmax)[:, g : g + 1],
                            scalar2=None,
                            op0=EQ,
                        )
                        # transpose x chunk into PSUM: xT[c, s]
                        nc.tensor.transpose(
                            xT[:, g, :], xb_t[:, g, :], ident_sb[:]
                        )
                        if fuse2:
                            nc.tensor.matmul(
                                ps[:],
                                oht[:, g, :],
                                rhs[:, g, :],
                                start=(it == 0 and g == 0),
                                stop=(it == iters - 1 and g == G - 1),
                            )
                    if fuse and not fuse2:
                        for g in range(G_):
                            nc.tensor.matmul(
                                ps[:],
                                oht[:, g, :],
                                rhs[:, g, :],
                                start=(it == 0 and g == 0),
                                stop=(it == iters - 1 and g == G - 1),
                            )
                    # exp on the transposed tile (PSUM -> SBUF)
                    EH = G_ // esplit
                    for h in range(esplit):
                        nc.scalar.activation(
                            eT[:, h * EH : (h + 1) * EH, :],
                            xT[:, h * EH : (h + 1) * EH, :],
                            mybir.ActivationFunctionType.Exp,
                        )
                    for g in range(G_):
                        # sumexp[s] = eT.T @ ones = row sums (PE, N=1)
                        nc.tensor.matmul(
                            se_ps[:, it * G_ + g : it * G_ + g + 1],
                            eT[:, g, :],
                            ones_sb[:],
                            start=True,
                            stop=True,
                        )
                        # segment-sum matmul: psum += oht.T @ [x|onehot(am)]
                        if fuse or fuse2:
                            pass
                        elif two_mm:
                            nc.tensor.matmul(
                                ps[:, 0:128],
                                oht[:, g, :],
                                xb_t[:, g, :],
                                start=(it == 0 and g == 0),
                                stop=(it == iters - 1 and g == G - 1),
                            )
                            nc.tensor.matmul(
                                ps[:, 128:256],
                                oht[:, g, :],
                                oham_t[:, g, :],
                                start=(it == 0 and g == 0),
                                stop=(it == iters - 1 and g == G - 1),
                            )
                        else:
                            nc.tensor.matmul(
                                ps[:],
                                oht[:, g, :],
                                rhs[:, g, :],
                                start=(it == 0 and g == 0),
                                stop=(it == iters - 1 and g == G - 1),
                            )
                q_sb = work.tile([128, 256], f32, tag="q")
                nc.scalar.copy(q_sb[:], ps[:])
                nc.sync.dma_start(q_out.ap()[b], q_sb[:])
                se_sb = work.tile([128, iters * G_], f32, tag="sesb")
                nc.scalar.copy(se_sb[:], se_ps[:])
                nc.sync.dma_start(se_out.ap()[b], se_sb[:])

    nc.compile()
    return nc


def _get_nc(b_loc=B_LOC, iters=ITERS):
    key = (b_loc, iters)
    if key not in _cache:
        _cache[key] = _build(b_loc, iters)
    return _cache[key]


_BF16 = ml_dtypes.bfloat16
_IDENT = np.eye(128).astype(_BF16)
_EYE = np.eye(128).astype(_BF16)
last_results = None


def _run_device(predicted, target):
    """predicted [B,S,C] f32, target [B,S] int -> (q [B,128,256], se [B,S]) float64"""
    from concourse.bass_utils import run_bass_kernel_spmd

    nc = _get_nc()
    xb = predicted.astype(_BF16)
    ohtb = _EYE[target.astype(np.int64)]
    in_maps = []
    for core in range(NCORES):
        b0 = core * B_LOC
        in_maps.append(
            {
                "predicted": np.ascontiguousarray(xb[b0 : b0 + B_LOC]),
                "oht_bf16": np.ascontiguousarray(ohtb[b0 : b0 + B_LOC]),
                "ident_bf16": _IDENT,
            }
        )
    global last_results
    last_results = run_bass_kernel_spmd(
        nc, in_maps, core_ids=list(range(NCORES))
    )
    q = np.concatenate([r["q_out"] for r in last_results.results], axis=0)
    se = np.concatenate([r["se_out"] for r in last_results.results], axis=0)
    # se[b, p, it*G+g] -> sumexp[b, s] with s = it*(G*128) + p*G + g
    se = (
        se.reshape(B, 128, ITERS, G)
        .transpose(0, 2, 1, 3)
        .reshape(B, S)
    )
    return q.astype(np.float64), se.astype(np.float64)


def kernel(predicted, target):
    predicted = np.asarray(predicted)
    target = np.asarray(target)
    in_dtype = predicted.dtype
    q, se = _run_device(predicted.astype(np.float32, copy=False), target)

    total_cipher = 0.0
    total_nz = 0
    total_gather = 0.0
    for b in range(B):
        Q = q[b, :, 0:128]          # [j, c] segment sums of x (bf16 inputs)
        counts = q[b, :, 128:256]   # [j, c] argmax histogram
        t_b = target[b].astype(np.int64)
        lse = np.log(se[b])
        n_eq = np.bincount(t_b, minlength=C).astype(np.float64)
        Lt = np.bincount(t_b, weights=lse, minlength=C)
        L = lse.sum()
        mode = np.argmax(counts, axis=1)
        P = Q.sum(axis=0)
        Qg = Q[np.arange(C), mode]
        Pg = P[mode]
        sum_all = L - Pg
        sum_eq = Lt - Qg
        sum_ne = sum_all - sum_eq
        ne_cnt = S - n_eq
        eq_mean = sum_eq / np.maximum(n_eq, 1.0)
        ne_mean = sum_ne / np.maximum(ne_cnt, 1.0)
        inv_ne = np.where(ne_cnt > 0, 1.0 / np.maximum(ne_mean, 1e-30), 0.0)
        cipher = np.where(n_eq > 0, 0.5 * eq_mean + 0.5 * inv_ne, 0.0)
        total_cipher += cipher.sum()
        total_nz += int((cipher != 0).sum())
        total_gather += Q[np.arange(C), np.arange(C)].sum()

    cipher_mean = total_cipher / max(total_nz, 1)
    nll = -total_gather / (B * S)
    out = 0.5 * cipher_mean + 0.5 * nll
    out_dtype = in_dtype if in_dtype in (np.float32, np.float64) else np.float32
    return np.asarray(out, dtype=out_dtype)


# revision 39
# speedup vs baseline: 1.1002x; 1.0237x over previous
"""Trainium2 Bass kernel for nn_CustomLoss_84043920048360.

Data-parallel over batch: 8 NeuronCores x 4 batches each, no collectives.

The loss reduces to per-batch segment-sums over positions s:
  Q[j, c]      = sum_{s: target[s]==j} x[s, c]
  counts[j, c] = sum_{s: target[s]==j} [argmax_c' x[s, c'] == c]
  sumexp[s]    = sum_c exp(x[s, c])

Device pipeline (per 128-position chunk, all-bf16 matmul path):
  - x ships as bf16, onehot(target) is built on the host (np.eye gather)
    and ships as bf16 -> total input DMA 16.8 MB/core.
  - DVE: rowmax (f32 out) + 6/8 argmax-onehot compares (bf16 4x mode);
    GPSIMD takes 2/8 compare chunks and the x copy into the matmul rhs
    (uint32-bitcast so the bf16 move halves its element count).
  - TensorE: ONE bf16 matmul per chunk, lhsT=onehot(target),
    rhs=[x | onehot(argmax)] (N=256), accumulated over 64 chunks in PSUM
    -> [Q | counts]. Plus a bf16 transpose of x into PSUM and an N=1
    ones-matmul on exp(xT) that computes sumexp on the PE.
  - ACT: exp (PSUM->SBUF, one op per 8 chunks) + PSUM evacuation copies.
The host does lse=log(sumexp), bincounts, mode=argmax(counts) (exact
tie-break), the cipher/nll formulas in float64, and the final combine.

Accuracy: counts/mode are exact except for bf16-argmax ties (~1.8% of
rows, washes out to ~2e-4 on the final scalar); Q/lse carry bf16 noise
(~1e-4). Measured end-to-end relative error vs the f32 reference: 1.6e-4.

Position mapping within a 1024-position block: s = it*1024 + p*8 + g
(p = SBUF partition, g = chunk-in-iter) so each partition's DMA is one
contiguous 2 KiB run.
"""

import numpy as np
import ml_dtypes

B, S, C = 32, 8192, 128
NCORES = 8
B_LOC = B // NCORES          # 4 batches per core
G = 16                       # chunks per iteration
CHUNK = 128                  # positions per chunk (matmul K)
ITERS = S // (G * CHUNK)     # 8 iterations per batch
NCHUNK = S // CHUNK          # 64 chunks per batch

_cache = {}


def _build(b_loc=B_LOC, iters=ITERS, n_pool=4, wbufs=4, pbufs=2, esplit=1, two_mm=False, fuse=True, fuse2=False, xbufs=0, rsplit=2, rmax_bf=False, g_ovr=0, csplit=1):
    import concourse.bacc as bacc
    import concourse.tile as tile
    from concourse import mybir

    f32 = mybir.dt.float32
    bf16 = mybir.dt.bfloat16
    u32 = mybir.dt.uint32
    G_ = g_ovr or G
    s_loc = iters * G_ * CHUNK

    nc = bacc.Bacc(
        "TRN2", target_bir_lowering=False, debug=False, num_devices=NCORES
    )
    pred = nc.dram_tensor("predicted", [b_loc, s_loc, C], bf16, kind="ExternalInput")
    oht_in = nc.dram_tensor("oht_bf16", [b_loc, s_loc, C], bf16, kind="ExternalInput")
    ident = nc.dram_tensor("ident_bf16", [128, 128], bf16, kind="ExternalInput")
    q_out = nc.dram_tensor("q_out", [b_loc, 128, 256], f32, kind="ExternalOutput")
    se_out = nc.dram_tensor(
        "se_out", [b_loc, 128, iters * G_], f32, kind="ExternalOutput"
    )

    # s = it*(G*128) + p*G + g
    pv = pred.ap().rearrange("b (i p g) c -> b i p g c", i=iters, p=128, g=G_)
    ov = oht_in.ap().rearrange("b (i p g) c -> b i p g c", i=iters, p=128, g=G_)

    AX = mybir.AxisListType.X
    EQ = mybir.AluOpType.is_equal

    with tile.TileContext(nc) as tc:
        with (
            tc.tile_pool(name="consts", bufs=1) as consts,
            tc.tile_pool(name="work", bufs=wbufs) as work,
            tc.tile_pool(name="psum", bufs=pbufs, space="PSUM") as psum,
            tc.tile_pool(name="psumx", bufs=(xbufs or pbufs), space="PSUM") as psumx,
        ):
            ident_sb = consts.tile([128, 128], bf16)
            nc.sync.dma_start(ident_sb[:], ident.ap())
            ones_sb = consts.tile([128, 1], bf16)
            nc.vector.memset(ones_sb[:], 1.0)

            for b in range(b_loc):
                ps = psum.tile([128, 256], f32, tag="ps")
                se_ps = psum.tile([128, iters * G_], f32, tag="se")
                for it in range(iters):
                    if two_mm:
                        oham_t = work.tile([128, G_, 128], bf16, tag="oham")
                        rhs = None
                    else:
                        rhs = work.tile([128, G_, 256], bf16, tag="rhs")
                    oht = work.tile([128, G_, 128], bf16, tag="oht")
                    eT = work.tile([128, G_, 128], bf16, tag="e")
                    rmax = work.tile([128, G_], bf16 if rmax_bf else f32, tag="rmax")
                    if rmax_bf:
                        rmax32 = work.tile([128, G_], f32, tag="rmax32")
                    xb_t = work.tile([128, G_, 128], bf16, tag="xbt")
                    xT = psumx.tile([128, G_, 128], bf16, tag="xT")

                    nc.sync.dma_start(xb_t[:], pv[b, it])
                    nc.sync.dma_start(oht[:], ov[b, it])
                    if not two_mm:
                        # x copy into the matmul rhs (bf16 moved as u32)
                        CH = G_ // csplit
                        for h in range(csplit):
                            nc.gpsimd.tensor_copy(
                                rhs[:, h * CH : (h + 1) * CH, 0:128].bitcast(u32),
                                xb_t[:, h * CH : (h + 1) * CH, :].bitcast(u32),
                            )
                    # rowmax over classes (bf16 in, f32 out)
                    RH = G_ // rsplit
                    for h in range(rsplit):
                        nc.vector.reduce_max(
                            rmax[:, h * RH : (h + 1) * RH],
                            xb_t[:, h * RH : (h + 1) * RH, :],
                            axis=AX,
                        )
                    if rmax_bf:
                        nc.vector.tensor_copy(rmax32[:], rmax[:])
                    for g in range(G_):
                        # onehot(argmax): bf16 compare, exact 0/1 out
                        eng = nc.gpsimd if g < n_pool else nc.vector
                        eng.tensor_scalar(
                            out=(oham_t[:, g, :] if two_mm else rhs[:, g, 128:256]),
                            in0=xb_t[:, g, :],
                            scalar1=(rmax32 if rmax_bf else rmax)[:, g : g + 1],
                            scalar2=None,
                            op0=EQ,
                        )
                        # transpose x chunk into PSUM: xT[c, s]
                        nc.tensor.transpose(
                            xT[:, g, :], xb_t[:, g, :], ident_sb[:]
                        )
                        if fuse2:
                            nc.tensor.matmul(
                                ps[:],
                                oht[:, g, :],
                                rhs[:, g, :],
                                start=(it == 0 and g == 0),
                                stop=(it == iters - 1 and g == G_ - 1),
                            )
                    if fuse and not fuse2:
                        for g in range(G_):
                            nc.tensor.matmul(
                                ps[:],
                                oht[:, g, :],
                                rhs[:, g, :],
                                start=(it == 0 and g == 0),
                                stop=(it == iters - 1 and g == G_ - 1),
                            )
                    # exp on the transposed tile (PSUM -> SBUF)
                    EH = G_ // esplit
                    for h in range(esplit):
                        nc.scalar.activation(
                            eT[:, h * EH : (h + 1) * EH, :],
                            xT[:, h * EH : (h + 1) * EH, :],
                            mybir.ActivationFunctionType.Exp,
                        )
                    for g in range(G_):
                        # sumexp[s] = eT.T @ ones = row sums (PE, N=1)
                        nc.tensor.matmul(
                            se_ps[:, it * G_ + g : it * G_ + g + 1],
                            eT[:, g, :],
                            ones_sb[:],
                            start=True,
                            stop=True,
                        )
                        # segment-sum matmul: psum += oht.T @ [x|onehot(am)]
                        if fuse or fuse2:
                            pass
                        elif two_mm:
                            nc.tensor.matmul(
                                ps[:, 0:128],
                                oht[:, g, :],
                                xb_t[:, g, :],
                                start=(it == 0 and g == 0),
                                stop=(it == iters - 1 and g == G_ - 1),
                            )
                            nc.tensor.matmul(
                                ps[:, 128:256],
                                oht[:, g, :],
                                oham_t[:, g, :],
                                start=(it == 0 and g == 0),
                                stop=(it == iters - 1 and g == G_ - 1),
                            )
                        else:
                            nc.tensor.matmul(
                                ps[:],
                                oht[:, g, :],
                                rhs[:, g, :],
                                start=(it == 0 and g == 0),
                                stop=(it == iters - 1 and g == G_ - 1),
                            )
                q_sb = work.tile([128, 256], f32, tag="q")
                nc.scalar.copy(q_sb[:], ps[:])
                nc.sync.dma_start(q_out.ap()[b], q_sb[:])
                se_sb = work.tile([128, iters * G_], f32, tag="sesb")
                nc.scalar.copy(se_sb[:], se_ps[:])
                nc.sync.dma_start(se_out.ap()[b], se_sb[:])

    nc.compile()
    return nc


def _get_nc(b_loc=B_LOC, iters=ITERS):
    key = (b_loc, iters)
    if key not in _cache:
        _cache[key] = _build(b_loc, iters)
    return _cache[key]


_BF16 = ml_dtypes.bfloat16
_IDENT = np.eye(128).astype(_BF16)
_EYE = np.eye(128).astype(_BF16)
last_results = None


def _run_device(predicted, target):
    """predicted [B,S,C] f32, target [B,S] int -> (q [B,128,256], se [B,S]) float64"""
    from concourse.bass_utils import run_bass_kernel_spmd

    nc = _get_nc()
    xb = predicted.astype(_BF16)
    ohtb = _EYE[target.astype(np.int64)]
    in_maps = []
    for core in range(NCORES):
        b0 = core * B_LOC
        in_maps.append(
            {
                "predicted": np.ascontiguousarray(xb[b0 : b0 + B_LOC]),
                "oht_bf16": np.ascontiguousarray(ohtb[b0 : b0 + B_LOC]),
                "ident_bf16": _IDENT,
            }
        )
    global last_results
    last_results = run_bass_kernel_spmd(
        nc, in_maps, core_ids=list(range(NCORES))
    )
    q = np.concatenate([r["q_out"] for r in last_results.results], axis=0)
    se = np.concatenate([r["se_out"] for r in last_results.results], axis=0)
    # se[b, p, it*G+g] -> sumexp[b, s] with s = it*(G*128) + p*G + g
    se = (
        se.reshape(B, 128, ITERS, G)
        .transpose(0, 2, 1, 3)
        .reshape(B, S)
    )
    return q.astype(np.float64), se.astype(np.float64)


def kernel(predicted, target):
    predicted = np.asarray(predicted)
    target = np.asarray(target)
    in_dtype = predicted.dtype
    q, se = _run_device(predicted.astype(np.float32, copy=False), target)

    total_cipher = 0.0
    total_nz = 0
    total_gather = 0.0
    for b in range(B):
        Q = q[b, :, 0:128]          # [j, c] segment sums of x (bf16 inputs)
        counts = q[b, :, 128:256]   # [j, c] argmax histogram
        t_b = target[b].astype(np.int64)
        lse = np.log(se[b])
        n_eq = np.bincount(t_b, minlength=C).astype(np.float64)
        Lt = np.bincount(t_b, weights=lse, minlength=C)
        L = lse.sum()
        mode = np.argmax(counts, axis=1)
        P = Q.sum(axis=0)
        Qg = Q[np.arange(C), mode]
        Pg = P[mode]
        sum_all = L - Pg
        sum_eq = Lt - Qg
        sum_ne = sum_all - sum_eq
        ne_cnt = S - n_eq
        eq_mean = sum_eq / np.maximum(n_eq, 1.0)
        ne_mean = sum_ne / np.maximum(ne_cnt, 1.0)
        inv_ne = np.where(ne_cnt > 0, 1.0 / np.maximum(ne_mean, 1e-30), 0.0)
        cipher = np.where(n_eq > 0, 0.5 * eq_mean + 0.5 * inv_ne, 0.0)
        total_cipher += cipher.sum()
        total_nz += int((cipher != 0).sum())
        total_gather += Q[np.arange(C), np.arange(C)].sum()

    cipher_mean = total_cipher / max(total_nz, 1)
    nll = -total_gather / (B * S)
    out = 0.5 * cipher_mean + 0.5 * nll
    out_dtype = in_dtype if in_dtype in (np.float32, np.float64) else np.float32
    return np.asarray(out, dtype=out_dtype)


# revision 41
# speedup vs baseline: 1.1180x; 1.0162x over previous
"""Trainium2 Bass kernel for nn_CustomLoss_84043920048360.

Data-parallel over batch: 8 NeuronCores x 4 batches each, no collectives.

The loss reduces to per-batch segment-sums over positions s:
  Q[j, c]      = sum_{s: target[s]==j} x[s, c]
  counts[j, c] = sum_{s: target[s]==j} [argmax_c' x[s, c'] == c]
  sumexp[s]    = sum_c exp(x[s, c])

Device pipeline (per 128-position chunk, all-bf16 matmul path):
  - x ships as bf16, onehot(target) is built on the host (np.eye gather)
    and ships as bf16 -> total input DMA 16.8 MB/core.
  - DVE: rowmax (f32 out) + 6/8 argmax-onehot compares (bf16 4x mode);
    GPSIMD takes 2/8 compare chunks and the x copy into the matmul rhs
    (uint32-bitcast so the bf16 move halves its element count).
  - TensorE: ONE bf16 matmul per chunk, lhsT=onehot(target),
    rhs=[x | onehot(argmax)] (N=256), accumulated over 64 chunks in PSUM
    -> [Q | counts]. Plus a bf16 transpose of x into PSUM and an N=1
    ones-matmul on exp(xT) that computes sumexp on the PE.
  - ACT: exp (PSUM->SBUF, one op per 8 chunks) + PSUM evacuation copies.
The host does lse=log(sumexp), bincounts, mode=argmax(counts) (exact
tie-break), the cipher/nll formulas in float64, and the final combine.

Accuracy: counts/mode are exact except for bf16-argmax ties (~1.8% of
rows, washes out to ~2e-4 on the final scalar); Q/lse carry bf16 noise
(~1e-4). Measured end-to-end relative error vs the f32 reference: 1.6e-4.

Position mapping within a 1024-position block: s = it*1024 + p*8 + g
(p = SBUF partition, g = chunk-in-iter) so each partition's DMA is one
contiguous 2 KiB run.
"""

import numpy as np
import ml_dtypes

B, S, C = 32, 8192, 128
NCORES = 8
B_LOC = B // NCORES          # 4 batches per core
G = 16                       # chunks per iteration
CHUNK = 128                  # positions per chunk (matmul K)
ITERS = S // (G * CHUNK)     # 8 iterations per batch
NCHUNK = S // CHUNK          # 64 chunks per batch

_cache = {}


def _build(b_loc=B_LOC, iters=ITERS, n_pool=4, wbufs=4, pbufs=2, esplit=1, two_mm=False, fuse=True, fuse2=False, xbufs=0, rsplit=2, rmax_bf=False, g_ovr=0, csplit=1, tr_first=False, mm_inter=True):
    import concourse.bacc as bacc
    import concourse.tile as tile
    from concourse import mybir

    f32 = mybir.dt.float32
    bf16 = mybir.dt.bfloat16
    u32 = mybir.dt.uint32
    G_ = g_ovr or G
    s_loc = iters * G_ * CHUNK

    nc = bacc.Bacc(
        "TRN2", target_bir_lowering=False, debug=False, num_devices=NCORES
    )
    pred = nc.dram_tensor("predicted", [b_loc, s_loc, C], bf16, kind="ExternalInput")
    oht_in = nc.dram_tensor("oht_bf16", [b_loc, s_loc, C], bf16, kind="ExternalInput")
    ident = nc.dram_tensor("ident_bf16", [128, 128], bf16, kind="ExternalInput")
    q_out = nc.dram_tensor("q_out", [b_loc, 128, 256], f32, kind="ExternalOutput")
    se_out = nc.dram_tensor(
        "se_out", [b_loc, 128, iters * G_], f32, kind="ExternalOutput"
    )

    # s = it*(G*128) + p*G + g
    pv = pred.ap().rearrange("b (i p g) c -> b i p g c", i=iters, p=128, g=G_)
    ov = oht_in.ap().rearrange("b (i p g) c -> b i p g c", i=iters, p=128, g=G_)

    AX = mybir.AxisListType.X
    EQ = mybir.AluOpType.is_equal

    with tile.TileContext(nc) as tc:
        with (
            tc.tile_pool(name="consts", bufs=1) as consts,
            tc.tile_pool(name="work", bufs=wbufs) as work,
            tc.tile_pool(name="psum", bufs=pbufs, space="PSUM") as psum,
            tc.tile_pool(name="psumx", bufs=(xbufs or pbufs), space="PSUM") as psumx,
        ):
            ident_sb = consts.tile([128, 128], bf16)
            nc.sync.dma_start(ident_sb[:], ident.ap())
            ones_sb = consts.tile([128, 1], bf16)
            nc.vector.memset(ones_sb[:], 1.0)

            for b in range(b_loc):
                ps = psum.tile([128, 256], f32, tag="ps")
                se_ps = psum.tile([128, iters * G_], f32, tag="se")
                for it in range(iters):
                    if two_mm:
                        oham_t = work.tile([128, G_, 128], bf16, tag="oham")
                        rhs = None
                    else:
                        rhs = work.tile([128, G_, 256], bf16, tag="rhs")
                    oht = work.tile([128, G_, 128], bf16, tag="oht")
                    eT = work.tile([128, G_, 128], bf16, tag="e")
                    rmax = work.tile([128, G_], bf16 if rmax_bf else f32, tag="rmax")
                    if rmax_bf:
                        rmax32 = work.tile([128, G_], f32, tag="rmax32")
                    xb_t = work.tile([128, G_, 128], bf16, tag="xbt")
                    xT = psumx.tile([128, G_, 128], bf16, tag="xT")

                    nc.sync.dma_start(xb_t[:], pv[b, it])
                    nc.sync.dma_start(oht[:], ov[b, it])
                    if not two_mm:
                        # x copy into the matmul rhs (bf16 moved as u32)
                        CH = G_ // csplit
                        for h in range(csplit):
                            nc.gpsimd.tensor_copy(
                                rhs[:, h * CH : (h + 1) * CH, 0:128].bitcast(u32),
                                xb_t[:, h * CH : (h + 1) * CH, :].bitcast(u32),
                            )
                    # rowmax over classes (bf16 in, f32 out)
                    RH = G_ // rsplit
                    for h in range(rsplit):
                        nc.vector.reduce_max(
                            rmax[:, h * RH : (h + 1) * RH],
                            xb_t[:, h * RH : (h + 1) * RH, :],
                            axis=AX,
                        )
                    if rmax_bf:
                        nc.vector.tensor_copy(rmax32[:], rmax[:])
                    if tr_first:
                        for g in range(G_):
                            nc.tensor.transpose(
                                xT[:, g, :], xb_t[:, g, :], ident_sb[:]
                            )
                    for g in range(G_):
                        # onehot(argmax): bf16 compare, exact 0/1 out
                        eng = nc.gpsimd if g < n_pool else nc.vector
                        eng.tensor_scalar(
                            out=(oham_t[:, g, :] if two_mm else rhs[:, g, 128:256]),
                            in0=xb_t[:, g, :],
                            scalar1=(rmax32 if rmax_bf else rmax)[:, g : g + 1],
                            scalar2=None,
                            op0=EQ,
                        )
                        if mm_inter:
                            nc.tensor.matmul(
                                ps[:],
                                oht[:, g, :],
                                rhs[:, g, :],
                                start=(it == 0 and g == 0),
                                stop=(it == iters - 1 and g == G_ - 1),
                            )
                        if not tr_first:
                            # transpose x chunk into PSUM: xT[c, s]
                            nc.tensor.transpose(
                                xT[:, g, :], xb_t[:, g, :], ident_sb[:]
                            )
                        if fuse2:
                            nc.tensor.matmul(
                                ps[:],
                                oht[:, g, :],
                                rhs[:, g, :],
                                start=(it == 0 and g == 0),
                                stop=(it == iters - 1 and g == G_ - 1),
                            )
                    if fuse and not fuse2 and not mm_inter:
                        for g in range(G_):
                            nc.tensor.matmul(
                                ps[:],
                                oht[:, g, :],
                                rhs[:, g, :],
                                start=(it == 0 and g == 0),
                                stop=(it == iters - 1 and g == G_ - 1),
                            )
                    # exp on the transposed tile (PSUM -> SBUF)
                    EH = G_ // esplit
                    for h in range(esplit):
                        nc.scalar.activation(
                            eT[:, h * EH : (h + 1) * EH, :],
                            xT[:, h * EH : (h + 1) * EH, :],
                            mybir.ActivationFunctionType.Exp,
                        )
                    for g in range(G_):
                        # sumexp[s] = eT.T @ ones = row sums (PE, N=1)
                        nc.tensor.matmul(
                            se_ps[:, it * G_ + g : it * G_ + g + 1],
                            eT[:, g, :],
                            ones_sb[:],
                            start=True,
                            stop=True,
                        )
                        # segment-sum matmul: psum += oht.T @ [x|onehot(am)]
                        if fuse or fuse2:
                            pass
                        elif two_mm:
                            nc.tensor.matmul(
                                ps[:, 0:128],
                                oht[:, g, :],
                                xb_t[:, g, :],
                                start=(it == 0 and g == 0),
                                stop=(it == iters - 1 and g == G_ - 1),
                            )
                            nc.tensor.matmul(
                                ps[:, 128:256],
                                oht[:, g, :],
                                oham_t[:, g, :],
                                start=(it == 0 and g == 0),
                                stop=(it == iters - 1 and g == G_ - 1),
                            )
                        else:
                            nc.tensor.matmul(
                                ps[:],
                                oht[:, g, :],
                                rhs[:, g, :],
                                start=(it == 0 and g == 0),
                                stop=(it == iters - 1 and g == G_ - 1),
                            )
                q_sb = work.tile([128, 256], f32, tag="q")
                nc.scalar.copy(q_sb[:], ps[:])
                nc.sync.dma_start(q_out.ap()[b], q_sb[:])
                se_sb = work.tile([128, iters * G_], f32, tag="sesb")
                nc.scalar.copy(se_sb[:], se_ps[:])
                nc.sync.dma_start(se_out.ap()[b], se_sb[:])

    nc.compile()
    return nc


def _get_nc(b_loc=B_LOC, iters=ITERS):
    key = (b_loc, iters)
    if key not in _cache:
        _cache[key] = _build(b_loc, iters)
    return _cache[key]


_BF16 = ml_dtypes.bfloat16
_IDENT = np.eye(128).astype(_BF16)
_EYE = np.eye(128).astype(_BF16)
last_results = None


def _run_device(predicted, target):
    """predicted [B,S,C] f32, target [B,S] int -> (q [B,128,256], se [B,S]) float64"""
    from concourse.bass_utils import run_bass_kernel_spmd

    nc = _get_nc()
    xb = predicted.astype(_BF16)
    ohtb = _EYE[target.astype(np.int64)]
    in_maps = []
    for core in range(NCORES):
        b0 = core * B_LOC
        in_maps.append(
            {
                "predicted": np.ascontiguousarray(xb[b0 : b0 + B_LOC]),
                "oht_bf16": np.ascontiguousarray(ohtb[b0 : b0 + B_LOC]),
                "ident_bf16": _IDENT,
            }
        )
    global last_results
    last_results = run_bass_kernel_spmd(
        nc, in_maps, core_ids=list(range(NCORES))
    )
    q = np.concatenate([r["q_out"] for r in last_results.results], axis=0)
    se = np.concatenate([r["se_out"] for r in last_results.results], axis=0)
    # se[b, p, it*G+g] -> sumexp[b, s] with s = it*(G*128) + p*G + g
    se = (
        se.reshape(B, 128, ITERS, G)
        .transpose(0, 2, 1, 3)
        .reshape(B, S)
    )
    return q.astype(np.float64), se.astype(np.float64)


def kernel(predicted, target):
    predicted = np.asarray(predicted)
    target = np.asarray(target)
    in_dtype = predicted.dtype
    q, se = _run_device(predicted.astype(np.float32, copy=False), target)

    total_cipher = 0.0
    total_nz = 0
    total_gather = 0.0
    for b in range(B):
        Q = q[b, :, 0:128]          # [j, c] segment sums of x (bf16 inputs)
        counts = q[b, :, 128:256]   # [j, c] argmax histogram
        t_b = target[b].astype(np.int64)
        lse = np.log(se[b])
        n_eq = np.bincount(t_b, minlength=C).astype(np.float64)
        Lt = np.bincount(t_b, weights=lse, minlength=C)
        L = lse.sum()
        mode = np.argmax(counts, axis=1)
        P = Q.sum(axis=0)
        Qg = Q[np.arange(C), mode]
        Pg = P[mode]
        sum_all = L - Pg
        sum_eq = Lt - Qg
        sum_ne = sum_all - sum_eq
        ne_cnt = S - n_eq
        eq_mean = sum_eq / np.maximum(n_eq, 1.0)
        ne_mean = sum_ne / np.maximum(ne_cnt, 1.0)
        inv_ne = np.where(ne_cnt > 0, 1.0 / np.maximum(ne_mean, 1e-30), 0.0)
        cipher = np.where(n_eq > 0, 0.5 * eq_mean + 0.5 * inv_ne, 0.0)
        total_cipher += cipher.sum()
        total_nz += int((cipher != 0).sum())
        total_gather += Q[np.arange(C), np.arange(C)].sum()

    cipher_mean = total_cipher / max(total_nz, 1)
    nll = -total_gather / (B * S)
    out = 0.5 * cipher_mean + 0.5 * nll
    out_dtype = in_dtype if in_dtype in (np.float32, np.float64) else np.float32
    return np.asarray(out, dtype=out_dtype)
